# revision 26
# baseline (speedup 1.0000x reference)
"""EvolveGCN forward on 8 Trainium2 NeuronCores.

Strategy (SPMD, one program on 8 cores):
  - Nodes sharded across cores (12500/core, padded to 12544 = 98 blocks of 128).
  - Edges partitioned by destination row; scatter stays core-local via
    one-hot matmul accumulation in PSUM (dest block = 128 psum partitions).
  - Source features gathered per-edge from a replicated table in core-local
    HBM with dma_gather. Gathers round-robin the 4 SWDGE queues so four
    Q7 core pairs generate descriptors concurrently (3.1x vs one queue).
  - Feature table is bf16 padded to 128 cols (256B rows: dma_gather's
    minimum elem/stride). Only cols 0:64 are real; the gathered pad half
    is never read. bf16 operands let the one-hot matmuls use FWL weight
    loads and halve DVE one-hot build cost.
  - Table quarters are LOCAL row chunks (3200/3200/3072/3072 per core) so
    each quarter's table is produced by its own AllGather chunk; AG chunks
    are issued as soon as their producer blocks are stored, overlapping
    collectives with gather/compute.
  - GCN norm folded as: t = relu(h@W+b) * dinv[node]  (source fold, on store)
    and h_block = psum * dinv[dest] (dest fold, on PSUM evacuation).

Host side: edge bucketing by (core, dest block, source chunk), degree/
dinv computation, x transpose, and final output assembly.
"""

import os
import numpy as np
import ml_dtypes

# ---- problem constants (hardcoded per contract) ----
N = 100000
E = 1600000
F_IN = 128
H = 64
HP = 128               # padded feature row (bf16 -> 256B = gather minimum)
NCORES = 8
SHARD = 12500          # real nodes per core
SHARD_PAD = 12544      # = 98 * 128
B = SHARD_PAD // 128   # 98 dest blocks per core
NQ = 4                 # table quarters (per-core local row ranges)
QS = [0, 3200, 6400, 9472, 12544]          # local quarter boundaries
CH = [3200, 3200, 3072, 3072]              # local quarter sizes
GQR = [8 * c for c in CH]                  # global quarter table rows
# One AllGather per quarter (splitting finer regressed: collective fixed
# costs outweigh the overlap gain). Sizes in 128-blocks:
SCB = [25, 25, 24, 24]                     # AG chunks (sum = 98)
SCE = [25, 50, 74, 98]                     # cumulative block ends
GMERGE = 4                                 # dest blocks per merged gather call
NQUEUES = 4                                # SWDGE queues for gather gen
NGBUF = 10                                 # gather buffers in flight

_BUILD_CACHE = {}
LAST_RESULTS = None     # BassKernelResults of the most recent run (for test harness)


def _f32_to_bf16_bits(a):
    """Round-to-nearest-even f32 -> bf16 (exact for small ints used here)."""
    return a.astype(ml_dtypes.bfloat16)


def _preprocess_edges(edge_index):
    """Bucket undirected edges by (core, dest block, source chunk).

    Per-(block, chunk) slot counts: jq_bq = max over cores of
    ceil(count/128) (shared across cores so the SPMD program has uniform
    loop bounds).  Layouts are concatenated along the free axis in the
    merged (block-group, chunk, block-in-group) order:
      colidx_i16: [NC, 128, 8*TOTJ] int16 - dma_gather index tiles
                  (16-partition wrap, replicated through 128 partitions)
      lrow_bf16:  [NC, 128, TOTJ] bf16 - local dest row per slot,
                  -1 for padding slots
      dinv:       [N] float32 - deg^-0.5 (0 for isolated nodes)
      jq_tab:     [B, NQ] int - slots/128 per group
    """
    e0 = np.asarray(edge_index[0], dtype=np.int64)
    e1 = np.asarray(edge_index[1], dtype=np.int64)
    row = np.concatenate([e0, e1]).astype(np.int32)
    col = np.concatenate([e1, e0]).astype(np.int32)

    deg = np.bincount(row, minlength=N).astype(np.float32)
    with np.errstate(divide="ignore"):
        dinv = np.where(deg > 0, deg.astype(np.float32) ** -0.5, 0.0).astype(np.float32)

    core = row // SHARD
    r_local = row - core * SHARD
    block = r_local >> 7          # // 128
    lr = r_local & 127            # % 128
    k2 = col // SHARD
    local2 = col - k2 * SHARD
    quarter = np.searchsorted(QS, local2, side="right") - 1   # local quarter id
    qs_arr = np.asarray(QS[:-1], dtype=np.int64)
    ch_arr = np.asarray(CH, dtype=np.int64)
    c_local = k2 * ch_arr[quarter] + (local2 - qs_arr[quarter])

    # group id and stable ordering
    gid = ((core * B + block) * NQ + quarter).astype(np.int64)
    order_e = np.argsort(gid, kind="stable")
    gid_s = gid[order_e]
    lr_s = lr[order_e].astype(np.float32)
    cl_s = c_local[order_e].astype(np.int16)

    ngroups = NCORES * B * NQ
    counts = np.bincount(gid_s, minlength=ngroups)
    # per-(b, q) slot count, maxed over cores for SPMD-uniform loop bounds
    jq_tab = np.ceil(counts.reshape(NCORES, B, NQ).max(axis=0) / 128
                     ).astype(np.int64)                      # [B, NQ]
    np.maximum(jq_tab, 1, out=jq_tab)
    TOTJ = int(jq_tab.sum())

    starts = np.zeros(ngroups, dtype=np.int64)
    starts[1:] = np.cumsum(counts)[:-1]
    # offset of each edge within its group
    off = np.arange(len(gid_s), dtype=np.int64) - starts[gid_s]

    # per-group slot base within the concatenated (bg, q, b-in-group) layout
    order = []
    for b0 in range(0, B, GMERGE):
        for q in range(NQ):
            for b in range(b0, min(b0 + GMERGE, B)):
                order.append(b * NQ + q)
    slot_base = np.zeros(B * NQ, dtype=np.int64)
    acc = 0
    for g in order:
        slot_base[g] = acc
        acc += int(jq_tab.reshape(-1)[g]) * 128

    # flat slot arrays per core; pad idx -> 0 (gathers row 0, zeroed by the
    # lrow=-1 one-hot; trailing -1 indices would skip descriptor generation
    # but crash the NRT worker, so padding stays at idx 0)
    colidx_flat = np.zeros((NCORES, TOTJ * 128), dtype=np.int16)
    lrow_flat = np.full((NCORES, TOTJ * 128), -1.0, dtype=np.float32)
    core_s = gid_s // (B * NQ)
    grp_s = gid_s % (B * NQ)
    pos = slot_base[grp_s] + off
    colidx_flat[core_s, pos] = cl_s
    lrow_flat[core_s, pos] = lr_s

    # per-group dma_gather index layout: slot i -> [16*rep + i%16, i//16];
    # lrow layout: slot i = j*128 + p -> [p, j]
    colidx_i16 = np.zeros((NCORES, 128, 8 * TOTJ), dtype=np.int16)
    lrow_f32 = np.full((NCORES, 128, TOTJ), -1.0, dtype=np.float32)
    jqs = jq_tab.reshape(-1)
    for g in order:
        jq = int(jqs[g])
        s0 = slot_base[g]
        o = s0 // 128
        ci = colidx_flat[:, s0:s0 + jq * 128].reshape(NCORES, 8 * jq, 16)
        ci = ci.transpose(0, 2, 1)                        # [NC, 16, 8jq]
        colidx_i16[:, :, 8 * o:8 * (o + jq)] = np.tile(ci, (1, 8, 1))
        lw = lrow_flat[:, s0:s0 + jq * 128].reshape(NCORES, jq, 128)
        lrow_f32[:, :, o:o + jq] = lw.transpose(0, 2, 1)

    return colidx_i16, _f32_to_bf16_bits(lrow_f32), dinv, jq_tab


def _build(jq_tab):
    """Build + compile the SPMD Bass program."""
    import concourse.bass as bass
    import concourse.mybir as mybir
    import concourse.tile as tile
    from concourse import bacc
    from concourse.masks import make_identity

    fp32 = mybir.dt.float32
    bf16 = mybir.dt.bfloat16
    i16 = mybir.dt.int16

    jqs = [[int(jq_tab[b][q]) for q in range(NQ)] for b in range(B)]
    TOTJ = sum(sum(r) for r in jqs)
    # free-axis offset (in J units) of each (b, q) group, in the merged
    # (bg, q, b-in-group) order used by _preprocess_edges
    joff = [[0] * NQ for _ in range(B)]
    acc_j = 0
    for b0 in range(0, B, GMERGE):
        for q in range(NQ):
            for b in range(b0, min(b0 + GMERGE, B)):
                joff[b][q] = acc_j
                acc_j += jqs[b][q]
    # max summed J per merged call (gather buffer size)
    JGMAX = max(
        sum(jqs[b][q] for b in range(b0, min(b0 + GMERGE, B)))
        for b0 in range(0, B, GMERGE) for q in range(NQ)
    )

    nc = bacc.Bacc("TRN2", target_bir_lowering=False, debug=False,
                   num_devices=NCORES, num_swdge_queues=NQUEUES)

    # ---- I/O tensors (per-core data via in_maps) ----
    xT = nc.dram_tensor("xT", [F_IN, SHARD_PAD], fp32, kind="ExternalInput").ap()
    W_in = nc.dram_tensor("W_in", [F_IN, H], fp32, kind="ExternalInput").ap()
    W1 = nc.dram_tensor("W1", [H, H], fp32, kind="ExternalInput").ap()
    W2 = nc.dram_tensor("W2", [H, H], fp32, kind="ExternalInput").ap()
    W_out = nc.dram_tensor("W_out", [H, H], fp32, kind="ExternalInput").ap()
    b_in = nc.dram_tensor("b_in", [H, 1], fp32, kind="ExternalInput").ap()
    b1 = nc.dram_tensor("b1", [H, 1], fp32, kind="ExternalInput").ap()
    b2 = nc.dram_tensor("b2", [H, 1], fp32, kind="ExternalInput").ap()
    b_out = nc.dram_tensor("b_out", [H, 1], fp32, kind="ExternalInput").ap()
    dinv_cols = nc.dram_tensor("dinv_cols", [128, B], fp32, kind="ExternalInput").ap()
    iota_in = nc.dram_tensor("iota_in", [128, 128], bf16, kind="ExternalInput").ap()
    colidx = nc.dram_tensor("colidx", [128, 8 * TOTJ], i16, kind="ExternalInput").ap()
    lrow = nc.dram_tensor("lrow", [128, TOTJ], bf16, kind="ExternalInput").ap()
    outT = nc.dram_tensor("outT", [H, SHARD_PAD], fp32, kind="ExternalOutput").ap()

    # internal DRAM. The AllGather moves compact [*, H] bf16 rows (1/4 the
    # padded-table bytes); a local dram->dram expand DMA then writes them
    # into the 256B-stride padded gather tables.
    t_own1 = nc.dram_tensor("t_own1", [SHARD_PAD, H], bf16)
    t_own2 = nc.dram_tensor("t_own2", [SHARD_PAD, H], bf16)
    t_ag1 = [nc.dram_tensor(f"t_ag1_c{c}", [GQR[c], H], bf16,
                            addr_space="Shared") for c in range(NQ)]
    t_ag2 = [nc.dram_tensor(f"t_ag2_c{c}", [GQR[c], H], bf16,
                            addr_space="Shared") for c in range(NQ)]
    t_full1 = [nc.dram_tensor(f"t_full1_c{c}", [GQR[c], HP], bf16)
               for c in range(NQ)]
    t_full2 = [nc.dram_tensor(f"t_full2_c{c}", [GQR[c], HP], bf16)
               for c in range(NQ)]

    groups = [list(range(NCORES))]

    def emit_ag(t_own_t, t_ag_t, t_full_t, sc):
        nc.gpsimd.collective_compute(
            "AllGather", mybir.AluOpType.bypass, replica_groups=groups,
            ins=[t_own_t[QS[sc]:QS[sc + 1], :]], outs=[t_ag_t[sc][:]],
        )
        # expand compact AG rows into the 256B-stride gather table
        # (ACT-engine HWDGE so it doesn't queue behind idx loads on Sync)
        nc.scalar.dma_start(t_full_t[sc][:, :H], t_ag_t[sc][:])

    with tile.TileContext(nc) as tc:
        with tc.tile_pool(name="const", bufs=1) as cpool:
            # constants resident for the whole kernel
            iota_t = cpool.tile([128, 128], bf16)
            nc.sync.dma_start(iota_t[:], iota_in[:])
            ident = cpool.tile([128, 128], fp32)
            make_identity(nc, ident[:])
            w_in_t = cpool.tile([F_IN, H], fp32)
            nc.sync.dma_start(w_in_t[:], W_in[:])
            w1_t = cpool.tile([H, H], fp32)
            nc.sync.dma_start(w1_t[:], W1[:])
            w2_t = cpool.tile([H, H], fp32)
            nc.sync.dma_start(w2_t[:], W2[:])
            w_out_t = cpool.tile([H, H], fp32)
            nc.sync.dma_start(w_out_t[:], W_out[:])
            bin_t = cpool.tile([H, 1], fp32)
            nc.sync.dma_start(bin_t[:], b_in[:])
            b1_t = cpool.tile([H, 1], fp32)
            nc.sync.dma_start(b1_t[:], b1[:])
            b2_t = cpool.tile([H, 1], fp32)
            nc.sync.dma_start(b2_t[:], b2[:])
            bout_t = cpool.tile([H, 1], fp32)
            nc.sync.dma_start(bout_t[:], b_out[:])
            dinv_t = cpool.tile([128, B], fp32)
            nc.sync.dma_start(dinv_t[:], dinv_cols[:])

            NSC = len(SCB)              # 8 AG sub-chunks per layer

            # ---- Phase A: t1 = relu((x@W_in + b_in)@W1 + b1) * dinv ----
            T = 512
            with tc.tile_pool(name="xf", bufs=3) as xf, \
                 tc.tile_pool(name="xfp", bufs=2, space="PSUM") as xfp:
                next_ag = 0
                pos = 0
                while pos < SHARD_PAD:
                    n = min(T, SHARD_PAD - pos)
                    xt = xf.tile([F_IN, T], fp32, tag="xt")
                    nc.sync.dma_start(xt[:, :n], xT[:, pos:pos + n])
                    h0p = xfp.tile([H, T], fp32, tag="h0p")
                    nc.tensor.matmul(h0p[:, :n], lhsT=w_in_t[:], rhs=xt[:, :n],
                                     start=True, stop=True)
                    h0s = xf.tile([H, T], fp32, tag="h0s")
                    nc.vector.tensor_scalar_add(h0s[:, :n], h0p[:, :n], bin_t[:, :1])
                    t1p = xfp.tile([H, T], fp32, tag="t1p")
                    nc.tensor.matmul(t1p[:, :n], lhsT=w1_t[:], rhs=h0s[:, :n],
                                     start=True, stop=True)
                    t1s = xf.tile([H, T], fp32, tag="t1s")
                    nc.scalar.activation(t1s[:, :n], t1p[:, :n],
                                         mybir.ActivationFunctionType.Relu,
                                         bias=b1_t[:, :1], scale=1.0)
                    # transpose to node-major in 128-col pieces, scale by dinv
                    for s in range(0, n, 128):
                        w = min(128, n - s)
                        bidx = (pos + s) // 128
                        tp = xfp.tile([128, H], fp32, tag="tp")
                        nc.tensor.transpose(tp[:w, :], t1s[:, s:s + w], ident[:H, :H])
                        tn = xf.tile([128, H], bf16, tag="tn")
                        nc.scalar.activation(tn[:w, :], tp[:w, :],
                                             mybir.ActivationFunctionType.Copy,
                                             bias=0.0, scale=dinv_t[:w, bidx:bidx + 1])
                        nc.sync.dma_start(t_own1[pos + s:pos + s + w, :], tn[:w, :])
                    pos += n
                    # issue AG sub-chunks once their blocks are stored (one
                    # tile late so the collective's sem wait doesn't
                    # head-of-line block the Pool queue while stores drain)
                    while next_ag < NSC and pos >= SCE[next_ag] * 128 + T:
                        emit_ag(t_own1, t_ag1, t_full1, next_ag)
                        next_ag += 1
                while next_ag < NSC:
                    emit_ag(t_own1, t_ag1, t_full1, next_ag)
                    next_ag += 1

            # ---- propagate + fused next transform, per block ----
            def propagate_layer(t_full, emit_epilogue, after_block=None):
                call_i = 0
                with tc.tile_pool(name="pg", bufs=3) as pg, \
                     tc.tile_pool(name="pgp", bufs=2, space="PSUM") as pgp, \
                     tc.tile_pool(name="accp", bufs=2, space="PSUM") as accp, \
                     tc.tile_pool(name="gbp", bufs=1) as gbp, \
                     tc.tile_pool(name="ohp", bufs=5) as ohp:
                    # persistent gather buffers, zeroed once
                    gatb = []
                    for i in range(NGBUF):
                        g = gbp.tile([128, JGMAX, HP], bf16, name=f"gatb{i}")
                        nc.vector.memset(g[:], 0.0)
                        gatb.append(g)
                    for b0 in range(0, B, GMERGE):
                        blocks = list(range(b0, min(b0 + GMERGE, B)))
                        jall = sum(jqs[b][q] for b in blocks for q in range(NQ))
                        og = joff[blocks[0]][0]
                        idx_t = pg.tile([128, 8 * jall], i16, tag="idx",
                                        padded_shape=[128, 8 * (JGMAX + 1) * NQ])
                        nc.sync.dma_start(idx_t[:], colidx[:, 8 * og:8 * (og + jall)])
                        lr_t = pg.tile([128, jall], bf16, tag="lr",
                                       padded_shape=[128, (JGMAX + 1) * NQ])
                        nc.sync.dma_start(lr_t[:], lrow[:, og:og + jall])
                        # all 4 block accumulators packed in one PSUM bank,
                        # double-buffered across groups. Chains into a shared
                        # bank must run sequentially per block (an interleaved
                        # chain's start resets bank accumulation state), so
                        # gathers + one-hots for all quarters are issued first
                        # and the matmul chains follow per block.
                        acc_t = accp.tile([128, GMERGE, H], fp32, tag="accs")
                        ohs = []
                        gats = []
                        for q in range(NQ):
                            jg = sum(jqs[b][q] for b in blocks)
                            lo = joff[blocks[0]][q] - og
                            gat = gatb[call_i % NGBUF]
                            nc.gpsimd.dma_gather(
                                gat[:, :jg, :], t_full[q][:],
                                idx_t[:, 8 * lo:8 * (lo + jg)],
                                jg * 128, jg * 128, HP,
                                elem_step=HP, single_packet=False,
                                queue_num=call_i % NQUEUES,
                            )
                            call_i += 1
                            oh = ohp.tile([128, jg, 128], bf16, tag="oh",
                                          padded_shape=[128, JGMAX, 128])
                            nc.vector.tensor_tensor(
                                out=oh[:],
                                in0=iota_t[:, None, :].to_broadcast([128, jg, 128]),
                                in1=lr_t[:, lo:lo + jg, None].to_broadcast(
                                    [128, jg, 128]),
                                op=mybir.AluOpType.is_equal,
                            )
                            ohs.append(oh)
                            gats.append(gat)
                        for bi, b in enumerate(blocks):
                            first = True
                            for q in range(NQ):
                                jrel = sum(jqs[bb][q] for bb in blocks[:bi])
                                for j in range(jrel, jrel + jqs[b][q]):
                                    nc.tensor.matmul(
                                        acc_t[:, bi, :], lhsT=ohs[q][:, j, :],
                                        rhs=gats[q][:, j, :H],
                                        start=first,
                                        stop=(q == NQ - 1
                                              and j == jrel + jqs[b][q] - 1),
                                    )
                                    first = False
                        for bi, b in enumerate(blocks):
                            emit_epilogue(b, acc_t[:, bi, :], pg, pgp)
                        if after_block is not None:
                            after_block(blocks[-1])

            def epilogue1(b, acc, pg, pgp):
                # h1 = acc * dinv_dest ; t2 = relu(h1@W2 + b2) * dinv -> t_own2
                h1s = pg.tile([128, H], fp32, tag="h1s")
                nc.scalar.activation(h1s[:], acc[:],
                                     mybir.ActivationFunctionType.Copy,
                                     bias=0.0, scale=dinv_t[:, b:b + 1])
                h1tp = pgp.tile([H, 128], fp32, tag="h1tp")
                nc.tensor.transpose(h1tp[:], h1s[:], ident[:])
                h1ts = pg.tile([H, 128], fp32, tag="h1ts")
                nc.scalar.copy(h1ts[:], h1tp[:])
                t2p = pgp.tile([H, 128], fp32, tag="t2p")
                nc.tensor.matmul(t2p[:], lhsT=w2_t[:], rhs=h1ts[:],
                                 start=True, stop=True)
                t2ts = pg.tile([H, 128], fp32, tag="t2ts")
                nc.scalar.activation(t2ts[:], t2p[:],
                                     mybir.ActivationFunctionType.Relu,
                                     bias=b2_t[:, :1], scale=1.0)
                t2np = pgp.tile([128, H], fp32, tag="t2np")
                nc.tensor.transpose(t2np[:], t2ts[:], ident[:H, :H])
                t2n = pg.tile([128, H], bf16, tag="t2n")
                nc.scalar.activation(t2n[:], t2np[:],
                                     mybir.ActivationFunctionType.Copy,
                                     bias=0.0, scale=dinv_t[:, b:b + 1])
                nc.sync.dma_start(t_own2[b * 128:(b + 1) * 128, :], t2n[:])

            def epilogue2(b, acc, pg, pgp):
                # h2 = acc * dinv_dest ; outT block = W_out.T @ h2.T + b_out
                h2s = pg.tile([128, H], fp32, tag="h2s")
                nc.scalar.activation(h2s[:], acc[:],
                                     mybir.ActivationFunctionType.Copy,
                                     bias=0.0, scale=dinv_t[:, b:b + 1])
                h2tp = pgp.tile([H, 128], fp32, tag="h2tp")
                nc.tensor.transpose(h2tp[:], h2s[:], ident[:])
                h2ts = pg.tile([H, 128], fp32, tag="h2ts")
                nc.scalar.copy(h2ts[:], h2tp[:])
                op = pgp.tile([H, 128], fp32, tag="op")
                nc.tensor.matmul(op[:], lhsT=w_out_t[:], rhs=h2ts[:],
                                 start=True, stop=True)
                os_ = pg.tile([H, 128], fp32, tag="os")
                # scalar engine, not vector: a vector op here would head-of-line
                # block the next group's IS_EQ one-hot builds on the DVE queue
                nc.scalar.activation(os_[:], op[:],
                                     mybir.ActivationFunctionType.Identity,
                                     bias=bout_t[:, :1], scale=1.0)
                nc.sync.dma_start(outT[:, b * 128:(b + 1) * 128], os_[:])

            ag2_state = {"next": 0}

            def after_block1(last_b):
                # one group late: the epilogue stores for the chunk have
                # drained by then, so the collective's sem wait doesn't
                # head-of-line block later gathers on the Pool queue
                while (ag2_state["next"] < NSC
                       and last_b + 1 >= SCE[ag2_state["next"]] + GMERGE):
                    emit_ag(t_own2, t_ag2, t_full2, ag2_state["next"])
                    ag2_state["next"] += 1

            propagate_layer(t_full1, epilogue1, after_block=after_block1)
            while ag2_state["next"] < NSC:
                emit_ag(t_own2, t_ag2, t_full2, ag2_state["next"])
                ag2_state["next"] += 1
            propagate_layer(t_full2, epilogue2)

    nc.compile()
    return nc


def kernel(**inputs):
    global LAST_RESULTS
    from concourse.bass_utils import run_bass_kernel_spmd

    x = np.asarray(inputs["x"], dtype=np.float32)
    edge_index = np.asarray(inputs["edge_index"])

    colidx, lrowv, dinv, jq_tab = _preprocess_edges(edge_index)

    key = jq_tab.tobytes()
    if key not in _BUILD_CACHE:
        _BUILD_CACHE[key] = _build(jq_tab)
    nc = _BUILD_CACHE[key]

    iota = np.ascontiguousarray(
        np.broadcast_to(np.arange(128, dtype=np.float32), (128, 128)))
    iota_bf16 = _f32_to_bf16_bits(iota)

    in_maps = []
    for k in range(NCORES):
        lo, hi = k * SHARD, (k + 1) * SHARD
        xT_k = np.zeros((F_IN, SHARD_PAD), np.float32)
        xT_k[:, :SHARD] = x[lo:hi].T
        dv = np.zeros(SHARD_PAD, np.float32)
        dv[:SHARD] = dinv[lo:hi]
        dinv_cols = np.ascontiguousarray(dv.reshape(B, 128).T)
        in_maps.append({
            "xT": xT_k,
            "W_in": np.asarray(inputs["W_in"], np.float32),
            "W1": np.asarray(inputs["W1"], np.float32),
            "W2": np.asarray(inputs["W2"], np.float32),
            "W_out": np.asarray(inputs["W_out"], np.float32),
            "b_in": np.asarray(inputs["b_in"], np.float32).reshape(H, 1),
            "b1": np.asarray(inputs["b1"], np.float32).reshape(H, 1),
            "b2": np.asarray(inputs["b2"], np.float32).reshape(H, 1),
            "b_out": np.asarray(inputs["b_out"], np.float32).reshape(H, 1),
            "dinv_cols": dinv_cols,
            "iota_in": iota_bf16,
            "colidx": colidx[k],
            "lrow": lrowv[k],
        })

    trace = bool(int(os.environ.get("GCN_TRACE", "0")))
    res = run_bass_kernel_spmd(nc, in_maps, core_ids=list(range(NCORES)),
                               trace=trace)
    LAST_RESULTS = res

    out = np.empty((N, H), np.float32)
    for k in range(NCORES):
        out[k * SHARD:(k + 1) * SHARD] = res.results[k]["outT"].T[:SHARD]
    return out
